# revision 6
# baseline (speedup 1.0000x reference)
"""Trainium2 Bass kernel v2 for nn_MitoticTransformerBlock.

Full causal attention (H=16, 2 heads/core) + soft-gated 2-expert FFN
(ff tensor-parallel, 512/core/expert), 8 NeuronCores.

Structure:
- 8 pipeline chunks of 512 tokens, software-pipelined by emission order:
  positions 0-7 run phase A (LN1 stats + QKV on raw x) interleaved with
  attention chunks 2 positions behind; positions 8-11 run the attention
  tail interleaved with FFN chunk PAIRS (all AllReduce latencies long
  absorbed by then). Engines execute their static streams in order, so
  emission interleave == cross-stream overlap.
- separate PSUM pools per stream (8 banks exactly: scores 2, PV-accum 2,
  gate/proj 2, expert-out/stats 2) - no cross-stream tag sharing.
- LN1 folded into the QKV matmuls via rank-1 correction rows (raw-x
  matmuls + (-mu) x rowsum and bias x (1/rstd) rank-1 updates).
- softmax probs and V in fp8e4m3 with a -3 exp shift (normalization
  cancels it); PV uses DoubleRow kt-pairs (2 key tiles per matmul).
- FFN out-projection in d-major (no PE transposes); x2/8 residual is
  accumulated into the expert PSUM via a scaled-identity matmul; partial
  sums ReduceScattered in bf16 per chunk (split in half for the last two
  chunks to shrink the tail); expert weights streamed from HBM per chunk.
- sigmoid gates via exp + reciprocal, rstd via Ln/Exp: the only ACT
  table set used is natural_log_exp (one table load, no thrash).
"""

import sys

sys.path.insert(0, "/opt/trn_rl_repo")

import numpy as np
import ml_dtypes

import concourse.bass as bass
import concourse.tile as tile
import concourse.mybir as mybir
from concourse import bacc
from concourse.bass_utils import run_bass_kernel_spmd
from concourse.masks import make_identity

F32 = mybir.dt.float32
BF16 = mybir.dt.bfloat16
FP8 = mybir.dt.float8e4
AF = mybir.ActivationFunctionType
OP = mybir.AluOpType
PM = mybir.MatmulPerfMode
NPBF16 = ml_dtypes.bfloat16
NPFP8 = ml_dtypes.float8_e4m3

NCORES = 8
B, T, D, H, FF = 1, 4096, 1024, 16, 4096
HD = D // H          # 64
DB = D // 128        # 8 d-blocks
CH = 512             # pipeline chunk (tokens) == one q-chunk
NCH = T // CH        # 8
NKT = T // 128       # 32 key tiles
FFS = FF // NCORES   # 512 ff slice per core per expert
NFB = FFS // 128     # 4 ff blocks
LN_EPS = 1e-5
VW = HD + 1          # 65: v columns + ones column
SCL_G = 32.0         # fp8 scales for eg / ep / eo
SCL_P = 32.0
SCL_O = 32.0
SCL_H = 8.0          # extra scale on stored hg
INV_HG = SCL_H / (SCL_G * SCL_P)      # rg * ps_p * INV_HG -> stored hg
INV_E = 1.0 / (SCL_H * SCL_O)         # psum_E * INV_E -> true expert out

_COMPILED = None


def _build_nc():
    nc = bacc.Bacc("TRN2", target_bir_lowering=False, debug=False,
                   num_devices=NCORES)

    def din(name, shape, dt):
        return nc.dram_tensor(name, shape, dt, kind="ExternalInput").ap()

    xT = din("xT", [D, T], BF16)
    wq = din("wq", [128, DB, 128], BF16)
    wk = din("wk", [128, DB, 128], BF16)
    wv = din("wv", [128, DB, 128], BF16)
    rsq = din("rsq", [1, 128], BF16)
    rsk = din("rsk", [1, 128], BF16)
    rsv = din("rsv", [1, 128], BF16)
    bqr = din("bqr", [1, 128], BF16)
    bkr = din("bkr", [1, 128], BF16)
    wo = din("wo", [128, DB, 128], BF16)
    wobv = din("wobv", [128, DB], F32)
    ln2st = din("ln2st", [128, DB, 3], BF16)
    scn = din("scn", [2, 1], F32)
    negc2 = din("negc2", [2, 1], F32)
    masks = din("masks", [128, 4, 1024], BF16)
    sel2b = din("sel2b", [2, 2, 64], BF16)
    sele = din("sele", [2, 2, 128], BF16)
    egt = din("egt", [2, 128, DB, NFB * 128], BF16)   # [e][dp, db, fb*128+m]
    ept = din("ept", [2, 128, DB, NFB * 128], BF16)
    eot = din("eot", [2, 128, NFB, D], BF16)          # [e][ffp, fb, db*128+m]

    out_rows = nc.dram_tensor("out_rows", [NCH, 16, DB * CH], BF16,
                              kind="ExternalOutput").ap()

    with tile.TileContext(nc) as tc:
        const = tc.alloc_tile_pool(name="const", bufs=1)
        work = tc.alloc_tile_pool(name="work", bufs=2)       # attention work
        fwork = tc.alloc_tile_pool(name="fwork", bufs=2)     # ffn work
        wstr = tc.alloc_tile_pool(name="wstr", bufs=3)       # streamed weights
        chunkA = tc.alloc_tile_pool(name="chunkA", bufs=2)   # att chunk tiles
        chunkF = tc.alloc_tile_pool(name="chunkF", bufs=2)   # ffn chunk tiles
        psS = tc.alloc_tile_pool(name="psS", bufs=1, space="PSUM")
        psPV = tc.alloc_tile_pool(name="psPV", bufs=1, space="PSUM")
        psGP = tc.alloc_tile_pool(name="psGP", bufs=2, space="PSUM")
        psE = tc.alloc_tile_pool(name="psE", bufs=2, space="PSUM")
        dram = tc.alloc_tile_pool(name="dram", bufs=1, space="DRAM")

        dma = nc.sync.dma_start

        # ---- constants into SBUF ----
        wq_sb = const.tile([128, DB, 128], BF16); dma(out=wq_sb, in_=wq)
        wk_sb = const.tile([128, DB, 128], BF16); dma(out=wk_sb, in_=wk)
        wv_sb = const.tile([128, DB, 128], BF16); dma(out=wv_sb, in_=wv)
        rsq_sb = const.tile([1, 128], BF16); dma(out=rsq_sb, in_=rsq)
        rsk_sb = const.tile([1, 128], BF16); dma(out=rsk_sb, in_=rsk)
        rsv_sb = const.tile([1, 128], BF16); dma(out=rsv_sb, in_=rsv)
        bqr_sb = const.tile([1, 128], BF16); dma(out=bqr_sb, in_=bqr)
        bkr_sb = const.tile([1, 128], BF16); dma(out=bkr_sb, in_=bkr)
        wo_sb = const.tile([128, DB, 128], BF16); dma(out=wo_sb, in_=wo)
        wobv_sb = const.tile([128, DB], F32); dma(out=wobv_sb, in_=wobv)
        ln2_sb = const.tile([128, DB, 3], BF16); dma(out=ln2_sb, in_=ln2st)
        scn_sb = const.tile([2, 1], F32); dma(out=scn_sb, in_=scn)
        negc2_sb = const.tile([2, 1], F32); dma(out=negc2_sb, in_=negc2)
        mask_sb = const.tile([128, 4, 1024], BF16); dma(out=mask_sb, in_=masks)
        sel2_sb = const.tile([2, 2, 64], BF16); dma(out=sel2_sb, in_=sel2b)
        sele_sb = const.tile([2, 2, 128], BF16); dma(out=sele_sb, in_=sele)
        id32 = const.tile([128, 128], BF16)
        make_identity(nc, id32)
        nc.vector.tensor_scalar_mul(id32, id32, 1.0 / NCORES)
        eps2 = const.tile([2, 1], F32)
        nc.gpsimd.memset(eps2, LN_EPS)
        eps128 = const.tile([128, 1], F32)
        nc.gpsimd.memset(eps128, LN_EPS)
        ones128 = const.tile([128, 128], BF16)
        nc.gpsimd.memset(ones128, 1.0)

        q_sb = const.tile([128, T], BF16)
        k_sb = const.tile([128, T], BF16)
        # v in fp8 (head-dim cols 0:64, ones col 64, pad to 72 so the
        # DoubleRow kt-pair stride 144 is 16B-aligned)
        v_sb = const.tile([128, NKT, 2, 72], FP8)
        nc.gpsimd.memset(v_sb, 1.0)
        rstd_tok = const.tile([128, NKT], F32)
        neg3 = const.tile([128, 1], F32)
        nc.gpsimd.memset(neg3, -3.0)

        # ---- collective buffers ----
        yb, yr, pb, ro = [], [], [], []
        for i in range(NCH):
            yb.append(dram.tile([128, DB * CH], BF16, tag=f"yb{i}",
                                name=f"yb{i}"))
            yr.append(dram.tile([128, DB * CH], BF16, tag=f"yr{i}",
                                name=f"yr{i}", addr_space="Shared"))
            pb.append(dram.tile([128, DB * CH], BF16, tag=f"pb{i}",
                                name=f"pb{i}"))
            ro.append(dram.tile([16, DB * CH], BF16, tag=f"ro{i}",
                                name=f"ro{i}"))
        pbh, roh = {}, {}
        for i in (NCH - 4, NCH - 3, NCH - 2, NCH - 1):
            pbh[i] = [dram.tile([128, DB * CH // 2], BF16, tag=f"pbh{i}{hf}",
                                name=f"pbh{i}{hf}") for hf in range(2)]
            roh[i] = [dram.tile([16, DB * CH // 2], BF16, tag=f"roh{i}{hf}",
                                name=f"roh{i}{hf}") for hf in range(2)]
        rg = [list(range(NCORES))]

        # warm up the collective path with a tiny AllReduce (absorbs the
        # first-collective setup latency under phase A)
        wrm_in = dram.tile([2, 16], F32, tag="wrm_in", name="wrm_in")
        wrm_out = dram.tile([2, 16], F32, tag="wrm_out", name="wrm_out",
                            addr_space="Shared")
        wrm_sb = const.tile([2, 16], F32)
        nc.gpsimd.memset(wrm_sb, 1.0)
        dma(out=wrm_in, in_=wrm_sb)
        nc.gpsimd.collective_compute("AllReduce", OP.add, replica_groups=rg,
                                     ins=[wrm_in[:]], outs=[wrm_out[:]])

        # ---- phase A: LN1 stats + QKV over raw x (rank-1 LN folding) ----
        xA_pool = tc.alloc_tile_pool(name="xA", bufs=2)

        def gen_phaseA(tch):
            if True:
                    ts = slice(CH * tch, CH * tch + CH)
                    xA = xA_pool.tile([128, DB, CH], BF16, tag="xA",
                                      name="xA")
                    for db in range(DB):
                        dma(out=xA[:, db, :],
                            in_=xT[128 * db:128 * db + 128, ts])
                    # stats: sum and sum-of-squares via ones-matmul
                    ps_su = psGP.tile([128, 512], F32, tag="gp", name="psAs")
                    ps_sq = psGP.tile([128, 512], F32, tag="gp", name="psAq")
                    sq_all = work.tile([128, DB, CH], BF16, tag="sq", bufs=1,
                                       name="sq")
                    for db in range(DB):
                        nc.vector.tensor_mul(sq_all[:, db, :], xA[:, db, :],
                                             xA[:, db, :])
                        nc.tensor.matmul(ps_su, ones128, xA[:, db, :],
                                         start=(db == 0), stop=(db == DB - 1))
                    yield
                    for db in range(DB):
                        nc.tensor.matmul(ps_sq, ones128, sq_all[:, db, :],
                                         start=(db == 0), stop=(db == DB - 1))
                    mu = work.tile([128, CH], F32, tag="mu", bufs=1, name="mu")
                    nc.vector.tensor_scalar_mul(mu, ps_su, 1.0 / D)
                    mu2 = work.tile([128, CH], F32, tag="mu2", bufs=1,
                                    name="mu2")
                    nc.vector.tensor_mul(mu2, mu, mu)
                    var = work.tile([128, CH], F32, tag="var", bufs=1,
                                    name="var")
                    nc.vector.scalar_tensor_tensor(var, ps_sq,
                                                   1.0 / D, mu2,
                                                   OP.mult, OP.subtract)
                    nc.scalar.activation(var, var, AF.Ln, bias=eps128,
                                         scale=1.0)
                    rstdB = work.tile([128, CH], F32, tag="rstdB", bufs=1,
                                      name="rstdB")
                    nc.scalar.activation(rstdB, var, AF.Exp, scale=-0.5)
                    negc = work.tile([1, CH], BF16, tag="negc", bufs=1,
                                     name="negc")
                    nc.vector.tensor_scalar_mul(negc, mu[0:1, :], -1.0)
                    invr = work.tile([1, CH], BF16, tag="invr", bufs=1,
                                     name="invr")
                    nc.scalar.activation(invr, var[0:1, :], AF.Exp, scale=0.5)
                    rts = dram.tile([1, CH], F32, tag="rts", bufs=2,
                                    name=f"rts{tch}")
                    dma(out=rts, in_=rstdB[0:1, :])
                    dma(out=rstd_tok[:, 4 * tch:4 * tch + 4],
                        in_=rts[:].rearrange("o (kk p) -> (o p) kk", p=128))
                    yield
                    # q, k projections (raw x + rank-1 corrections)
                    for (wsb, rssb, bsb, dst) in ((wq_sb, rsq_sb, bqr_sb, q_sb),
                                                  (wk_sb, rsk_sb, bkr_sb, k_sb)):
                        ps_q = psGP.tile([128, 512], F32, tag="gp",
                                         name="ps_qk")
                        for db in range(DB):
                            nc.tensor.matmul(ps_q, wsb[:, db, :], xA[:, db, :],
                                             start=(db == 0), stop=False)
                        nc.tensor.matmul(ps_q, rssb, negc,
                                         start=False, stop=False)
                        nc.tensor.matmul(ps_q, bsb, invr,
                                         start=False, stop=True)
                        nc.vector.tensor_mul(dst[:, ts], ps_q, rstdB)
                        yield
                    # v projection, token-major (x tiles stationary)
                    for kt4 in range(4):
                        ktg = 4 * tch + kt4
                        tts = slice(128 * kt4, 128 * kt4 + 128)
                        ps_v = psE.tile([128, 512], F32, tag="e", name="ps_v")
                        for db in range(DB):
                            nc.tensor.matmul(ps_v[:, 0:128], xA[:, db, tts],
                                             wv_sb[:, db, :],
                                             start=(db == 0), stop=False)
                        nc.tensor.matmul(ps_v[:, 0:128],
                                         negc[:, tts], rsv_sb,
                                         start=False, stop=True)
                        nc.vector.tensor_scalar(
                            v_sb[:, ktg, :, 0:HD],
                            ps_v[:, 0:128].rearrange("p (h j) -> p h j", h=2),
                            rstd_tok[:, ktg:ktg + 1], None, OP.mult)
                    yield

        def gen_att(c):
            # attention for q-chunk c (tokens 512c .. 512c+512)
            if True:
                tqs0 = CH * c
                pvp = psPV.tile([VW, 1024], F32, tag="pv", name="pvp")
                nkts = 4 * c + 4
                p8 = None
                for kt in range(nkts):
                    kts = slice(128 * kt, 128 * kt + 128)
                    j = kt - 4 * c
                    off = 128 * j if j > 0 else 0
                    ps_s = psS.tile([128, 1024], F32, tag="s",
                                    name="ps_s")
                    if j < 0:
                        # in-range tile: fp8 probs, PV via DoubleRow kt-pairs
                        if kt % 2 == 0:
                            p8 = work.tile([128, 2, 1024], FP8, tag="p8",
                                           bufs=2, name="p8")
                        for h in range(2):
                            nc.tensor.matmul(
                                ps_s[:, 512 * h:512 * h + 512],
                                k_sb[64 * h:64 * h + 64, kts],
                                q_sb[64 * h:64 * h + 64,
                                     tqs0:tqs0 + 512],
                                start=True, stop=True,
                                tile_position=(64 * h, 0))
                        nc.scalar.activation(p8[:, kt % 2, :], ps_s, AF.Exp,
                                             bias=neg3, scale=1.0)
                        if kt % 2 == 1:
                            for h in range(2):
                                nc.tensor.matmul(
                                    pvp[:, 512 * h:512 * h + 512],
                                    v_sb[:, kt - 1:kt + 1, h, 0:VW],
                                    p8[:, :, 512 * h:512 * h + 512],
                                    start=(kt == 1), stop=False,
                                    skip_group_check=True,
                                    perf_mode=PM.DoubleRow)
                    else:
                        for h in range(2):
                            nc.tensor.matmul(
                                ps_s[:, 512 * h + off:512 * h + 512],
                                k_sb[64 * h:64 * h + 64, kts],
                                q_sb[64 * h:64 * h + 64,
                                     tqs0 + off:tqs0 + 512],
                                start=True, stop=True,
                                tile_position=(64 * h, 0))
                        p_sb = work.tile([128, 1024], BF16, tag="p", bufs=3,
                                         name="p_sb")
                        pass
                        if off:
                            s3 = ps_s.rearrange("p (h t) -> p h t",
                                                h=2)[:, :, off:512]
                            p3 = p_sb.rearrange("p (h t) -> p h t",
                                                h=2)[:, :, off:512]
                            m3 = mask_sb[:, j, :].rearrange(
                                "p (h t) -> p h t", h=2)[:, :, off:512]
                            nc.scalar.activation(p3, s3, AF.Exp, bias=neg3,
                                                 scale=1.0)
                            nc.vector.tensor_mul(p3, p3, m3)
                        else:
                            nc.scalar.activation(p_sb, ps_s, AF.Exp,
                                                 bias=neg3, scale=1.0)
                            nc.vector.tensor_mul(p_sb, p_sb, mask_sb[:, 0, :])
                        for h in range(2):
                            nc.tensor.matmul(
                                pvp[:, 512 * h + off:512 * h + 512],
                                v_sb[:, kt, h, 0:VW],
                                p_sb[:, 512 * h + off:512 * h + 512],
                                start=(kt == 0), stop=(kt == nkts - 1),
                                skip_group_check=True)
                    yield
                # softmax denominators -> normalized attn rows
                lrow = work.tile([2, 512], F32, tag="lrow", bufs=1,
                                 name="lrow")
                for h in range(2):
                    ltmp = work.tile([65, 512], F32, tag="ltmp", bufs=1,
                                     name="ltmp")
                    nc.scalar.copy(ltmp[64:65, :],
                                   pvp[HD:HD + 1, 512 * h:512 * h + 512])
                    dma(out=lrow[h:h + 1, :], in_=ltmp[64:65, :])
                lrec = work.tile([2, 512], F32, tag="lrec", bufs=1,
                                 name="lrec")
                nc.vector.reciprocal_approx_fast(lrec, lrow)
                lrecb = work.tile([2, 512], BF16, tag="lrecb", bufs=1,
                                  name="lrecb")
                nc.vector.tensor_copy(lrecb, lrec)
                ps_li = psS.tile([128, 1024], F32, tag="s", name="ps_li")
                for h in range(2):
                    nc.tensor.matmul(ps_li[0:64, 512 * h:512 * h + 512],
                                     sel2_sb[:, h, :], lrecb,
                                     start=True, stop=True,
                                     skip_group_check=True)
                at_sb = work.tile([128, 512], BF16, tag="at", bufs=1,
                                  name="at")
                atn1 = work.tile([64, 512], BF16, tag="atn1", bufs=1,
                                 name="atn1")
                li0 = work.tile([64, 512], BF16, tag="li0", bufs=1,
                                name="li0")
                nc.scalar.copy(li0, ps_li[0:64, 0:512])
                li1 = work.tile([64, 512], BF16, tag="li1", bufs=1,
                                name="li1")
                nc.scalar.copy(li1, ps_li[0:64, 512:1024])
                nc.vector.tensor_mul(at_sb[0:64, :], pvp[0:HD, 0:512], li0)
                nc.vector.tensor_mul(atn1, pvp[0:HD, 512:1024], li1)
                dma(out=at_sb[64:128, :], in_=atn1)
                yield
                # out-projection -> y^T (d-major) + folded v-bias
                yT_sb = chunkA.tile([128, DB, 512], BF16, tag="yT",
                                    name="yT")
                for dp in range(DB // 2):
                    ps_y = psS.tile([128, 1024], F32, tag="s", name="ps_y")
                    for half in range(2):
                        db = 2 * dp + half
                        nc.tensor.matmul(ps_y[:, 512 * half:512 * half + 512],
                                         wo_sb[:, db, :], at_sb,
                                         start=True, stop=True,
                                         skip_group_check=True)
                    for half in range(2):
                        db = 2 * dp + half
                        nc.scalar.activation(
                            yT_sb[:, db, :],
                            ps_y[:, 512 * half:512 * half + 512],
                            AF.Identity, bias=wobv_sb[:, db:db + 1],
                            scale=1.0)
                    yield
                dma(out=yb[c][:].rearrange("p (b t) -> p b t", b=DB),
                    in_=yT_sb)
                nc.gpsimd.collective_compute(
                    "AllReduce", OP.add, replica_groups=rg,
                    ins=[yb[c][:]], outs=[yr[c][:]])

        def gen_ffn(c):
            if True:
                ts = slice(CH * c, CH * c + CH)
                yTr = chunkF.tile([128, DB, CH], BF16, tag="yTr", name="yTr")
                dma(out=yTr, in_=yr[c][:].rearrange("p (b t) -> p b t", b=DB))
                x2T = chunkF.tile([128, DB, CH], BF16, tag="x2T", name="x2T")
                dma(out=x2T,
                    in_=xT[:, ts].rearrange("(b p) t -> p b t", p=128))
                nc.vector.tensor_add(x2T, x2T, yTr)
                yield

                # LN2 stats + router gates
                ps_st = psE.tile([128, 512], F32, tag="e", name="ps_st")
                for db in range(DB):
                    nc.tensor.matmul(ps_st[0:3, :], ln2_sb[:, db, :],
                                     x2T[:, db, :],
                                     start=(db == 0), stop=(db == DB - 1))
                ps_sq2 = psE.tile([128, 512], F32, tag="e", name="ps_sq2")
                for db in range(DB):
                    sq = fwork.tile([128, CH], BF16, tag="fsq", bufs=2,
                                    name="fsq")
                    nc.vector.tensor_mul(sq, x2T[:, db, :], x2T[:, db, :])
                    nc.tensor.matmul(ps_sq2[0:1, :], ln2_sb[:, db, 0:1], sq,
                                     start=(db == 0), stop=(db == DB - 1))
                yield
                stS = fwork.tile([3, 512], BF16, tag="stS", bufs=2, name="stS")
                nc.scalar.copy(stS, ps_st[0:3, :])
                sqS = fwork.tile([1, 512], BF16, tag="sqS", bufs=2, name="sqS")
                nc.scalar.copy(sqS, ps_sq2[0:1, :])
                sum2 = fwork.tile([2, 512], BF16, tag="sum2", bufs=2,
                                  name="sum2")
                dots = fwork.tile([2, 512], BF16, tag="dots", bufs=2,
                                  name="dots")
                sq2b = fwork.tile([2, 512], BF16, tag="sq2b", bufs=2,
                                  name="sq2b")
                for e in range(2):
                    dma(out=sum2[e:e + 1, :], in_=stS[0:1, :])
                    dma(out=dots[e:e + 1, :], in_=stS[1 + e:2 + e, :])
                    dma(out=sq2b[e:e + 1, :], in_=sqS[0:1, :])
                t1 = fwork.tile([2, 512], BF16, tag="t1", bufs=2, name="t1")
                nc.vector.tensor_mul(t1, sum2, sum2)
                nc.vector.scalar_tensor_tensor(t1, t1, -1.0 / D, sq2b,
                                               OP.mult, OP.add)
                nc.scalar.activation(t1, t1, AF.Ln, bias=eps2, scale=1.0 / D)
                t1r = fwork.tile([2, 512], BF16, tag="t1r", bufs=2, name="t1r")
                nc.scalar.activation(t1r, t1, AF.Exp, scale=-0.5)
                zr = fwork.tile([2, 512], BF16, tag="zr", bufs=2, name="zr")
                nc.vector.scalar_tensor_tensor(zr, sum2, scn_sb, dots,
                                               OP.mult, OP.add)
                nc.vector.tensor_mul(zr, zr, t1r)
                eb = fwork.tile([2, 512], F32, tag="eb", bufs=1, name="eb")
                nc.scalar.activation(eb, zr, AF.Exp, bias=negc2_sb, scale=-1.0)
                nc.vector.tensor_scalar_add(eb, eb, 1.0)
                gr = fwork.tile([2, 512], F32, tag="gr", bufs=1, name="gr")
                nc.vector.reciprocal_approx_fast(gr, eb)
                grb = fwork.tile([2, 512], BF16, tag="grb", bufs=1,
                                 name="grb")
                nc.vector.tensor_copy(grb, gr)
                gb = chunkF.tile([128, 2, CH], BF16, tag="gb", name="gb")
                for e in range(2):
                    ps_g = psE.tile([128, 512], F32, tag="e", name="ps_gb")
                    nc.tensor.matmul(ps_g, sele_sb[:, e, :], grb,
                                     start=True, stop=True)
                    nc.scalar.copy(gb[:, e, :], ps_g)
                yield

                # experts: hg = relu(x2@eg)*gate * (x2@ep), streamed weights
                hg = [chunkF.tile([128, NFB, CH], BF16, tag=f"hg{e}",
                                  name=f"hg{e}") for e in range(2)]
                for e in range(2):
                    for fb in range(NFB):
                        wgb = wstr.tile([128, DB, 128], BF16, tag="wg",
                                        name="wgb")
                        dma(out=wgb, in_=egt[e][:, :, 128 * fb:128 * fb + 128])
                        wpb = wstr.tile([128, DB, 128], BF16, tag="wp",
                                        name="wpb")
                        dma(out=wpb, in_=ept[e][:, :, 128 * fb:128 * fb + 128])
                        ps_g = psGP.tile([128, 512], F32, tag="gp",
                                         name="ps_eg")
                        for db in range(DB):
                            nc.tensor.matmul(ps_g, wgb[:, db, :],
                                             x2T[:, db, :],
                                             start=(db == 0),
                                             stop=(db == DB - 1))
                        ps_p = psGP.tile([128, 512], F32, tag="gp",
                                         name="ps_ep")
                        for db in range(DB):
                            nc.tensor.matmul(ps_p, wpb[:, db, :],
                                             x2T[:, db, :],
                                             start=(db == 0),
                                             stop=(db == DB - 1))
                        rg_t = fwork.tile([128, 512], BF16, tag="rg", bufs=2,
                                          name="rg")
                        nc.vector.scalar_tensor_tensor(rg_t, ps_g, 0.0,
                                                       gb[:, e, :],
                                                       OP.max, OP.mult)
                        nc.vector.tensor_mul(hg[e][:, fb, :], rg_t, ps_p)
                        yield

                # out-experts, d-major + x2/8 residual
                for db in range(DB):
                    wob = [wstr.tile([128, NFB, 128], BF16, tag=f"wo8{e}",
                                     name="wob") for e in range(2)]
                    for e in range(2):
                        dma(out=wob[e],
                            in_=eot[e][:, :, 128 * db:128 * db + 128])
                    ps_E = psE.tile([128, 512], F32, tag="e", name="ps_E")
                    nc.tensor.matmul(ps_E, id32, x2T[:, db, :],
                                     start=True, stop=False)
                    for e in range(2):
                        for fb in range(NFB):
                            nc.tensor.matmul(
                                ps_E, wob[e][:, fb, :], hg[e][:, fb, :],
                                start=False,
                                stop=(e == 1 and fb == NFB - 1))
                    po = fwork.tile([128, 512], BF16, tag="po", bufs=2,
                                    name="po")
                    nc.vector.tensor_copy(po, ps_E)
                    if c in pbh:
                        hf = db // 4
                        dma(out=pbh[c][hf][:, 512 * (db % 4):
                                           512 * (db % 4) + 512], in_=po)
                        if db % 4 == 3:
                            nc.gpsimd.collective_compute(
                                "ReduceScatter", OP.add, replica_groups=rg,
                                ins=[pbh[c][hf][:]], outs=[roh[c][hf][:]])
                            dma(out=out_rows[c][:, 2048 * hf:2048 * hf + 2048],
                                in_=roh[c][hf][:])
                            yield
                    else:
                        dma(out=pb[c][:, 512 * db:512 * db + 512], in_=po)
                        if db % 4 == 3:
                            yield
                if c not in pbh:
                    nc.gpsimd.collective_compute(
                        "ReduceScatter", OP.add, replica_groups=rg,
                        ins=[pb[c][:]], outs=[ro[c][:]])
                    dma(out=out_rows[c], in_=ro[c][:])

        def drain(g):
            for _ in g:
                pass

        def interleave(*gens, head=0):
            alive = list(gens)
            for _ in range(head):
                try:
                    next(alive[0])
                except StopIteration:
                    alive.remove(alive[0])
                    break
            while alive:
                for g in list(alive):
                    try:
                        next(g)
                    except StopIteration:
                        alive.remove(g)

        # region 1 (positions 0-7): phase A per token-chunk, attention
        # joining 2 positions behind; region 2 (8-11): attention tail with
        # paired FFN chunks (AR latencies long since absorbed).
        posg = [[] for _ in range(12)]
        for t in range(NCH):
            posg[t].append(gen_phaseA(t))
        for c in range(6):
            posg[c + 2].append(gen_att(c))
        posg[8].append(gen_att(6))
        posg[9].append(gen_att(7))
        posg[8] += [gen_ffn(0), gen_ffn(1)]
        posg[9] += [gen_ffn(2), gen_ffn(3)]
        posg[10] += [gen_ffn(4), gen_ffn(5)]
        posg[11] += [gen_ffn(6), gen_ffn(7)]
        for p, gens in enumerate(posg):
            interleave(*gens, head=(6 if p >= 9 else 0))
            if p == 7:
                xA_pool.release()

        for pl in (dram, psE, psGP, psPV, psS, chunkF, chunkA,
                   wstr, fwork, work, const):
            pl.release()

    nc.compile()
    return nc


def _prep_inputs(inputs):
    f32 = np.float32

    def np32(a):
        return np.asarray(a, dtype=f32)

    x = np32(inputs["x"])[0]                      # [T, D]
    ln1_w, ln1_b = np32(inputs["ln1_w"]), np32(inputs["ln1_b"])
    ln2_w, ln2_b = np32(inputs["ln2_w"]), np32(inputs["ln2_b"])
    Wq, Wk, Wv, Wo = (np32(inputs[k]) for k in ("Wq", "Wk", "Wv", "Wo"))
    router_w, router_b = np32(inputs["router_w"]), np32(inputs["router_b"])
    eg, ep, eo = np32(inputs["eg"]), np32(inputs["ep"]), np32(inputs["eo"])

    xT = np.ascontiguousarray(x.T).astype(NPBF16)          # [D, T]
    scale_q = 1.0 / np.sqrt(HD)

    rw_eff = router_w * ln2_w[None, :]                     # [2, D]
    S = rw_eff.sum(axis=1)
    c_e = router_b + router_w @ ln2_b
    scn = (-(S / D)).reshape(2, 1).astype(f32)
    negc2 = (-c_e).reshape(2, 1).astype(f32)

    ln2st = np.zeros((128, DB, 3), f32)
    rw_r = rw_eff.reshape(2, DB, 128)
    ln2st[:, :, 0] = 1.0
    ln2st[:, :, 1] = rw_r[0].T
    ln2st[:, :, 2] = rw_r[1].T

    m = np.zeros((128, 4, 512), f32)
    p_i = np.arange(128)[:, None]
    t_i = np.arange(512)[None, :]
    for j in range(4):
        m[:, j, :] = (t_i >= 128 * j + p_i)
    masks = np.concatenate([m, m], axis=2).astype(NPBF16)  # [128, 4, 1024]

    sel2b = np.zeros((2, 2, 64), NPBF16)
    sel2b[0, 0, :] = 1.0
    sel2b[1, 1, :] = 1.0
    sele = np.zeros((2, 2, 128), NPBF16)
    sele[0, 0, :] = 1.0
    sele[1, 1, :] = 1.0

    def stat_pack(Wsh):  # [128(m), D] -> [128(kp), DB, 128(m)]
        return np.ascontiguousarray(
            Wsh.T.reshape(DB, 128, 128).transpose(1, 0, 2))

    def q8(a):
        return np.clip(a, -240.0, 240.0).astype(NPFP8)

    in_maps = []
    for c in range(NCORES):
        hs = slice(128 * c, 128 * c + 128)
        Wq_sh = (Wq * ln1_w[None, :])[hs] * scale_q        # [128, D]
        Wk_sh = (Wk * ln1_w[None, :])[hs]
        Wv_sh = (Wv * ln1_w[None, :])[hs]
        bq = (Wq[hs] @ ln1_b) * scale_q
        bk = Wk[hs] @ ln1_b
        bv = Wv[hs] @ ln1_b
        Wo_sh = Wo[:, hs]                                  # [D, 128]
        wo_pack = np.ascontiguousarray(
            Wo_sh.reshape(DB, 128, 128).transpose(2, 0, 1))  # [i, db, m]
        wobv = (Wo_sh @ bv).reshape(DB, 128).T.astype(f32)   # [128, DB]

        fs = slice(FFS * c, FFS * c + FFS)
        egt = np.stack([
            eg[e][fs].T.reshape(DB, 128, NFB, 128)
            .transpose(1, 0, 2, 3).reshape(128, DB, NFB * 128)
            .astype(NPBF16) for e in range(2)])
        ept = np.stack([
            ep[e][fs].T.reshape(DB, 128, NFB, 128)
            .transpose(1, 0, 2, 3).reshape(128, DB, NFB * 128)
            .astype(NPBF16) for e in range(2)])
        eot = np.stack([
            np.ascontiguousarray(eo[e][:, fs].T)
            .reshape(NFB, 128, DB, 128)
            .transpose(1, 0, 2, 3).reshape(128, NFB, D)
            .astype(NPBF16) for e in range(2)])

        in_maps.append({
            "xT": xT,
            "wq": stat_pack(Wq_sh).astype(NPBF16),
            "wk": stat_pack(Wk_sh).astype(NPBF16),
            "wv": stat_pack(Wv_sh).astype(NPBF16),
            "rsq": Wq_sh.sum(1).reshape(1, 128).astype(NPBF16),
            "rsk": Wk_sh.sum(1).reshape(1, 128).astype(NPBF16),
            "rsv": Wv_sh.sum(1).reshape(1, 128).astype(NPBF16),
            "bqr": bq.reshape(1, 128).astype(NPBF16),
            "bkr": bk.reshape(1, 128).astype(NPBF16),
            "wo": wo_pack.astype(NPBF16),
            "wobv": wobv,
            "ln2st": ln2st.astype(NPBF16),
            "scn": scn, "negc2": negc2, "masks": masks,
            "sel2b": sel2b, "sele": sele,
            "egt": egt, "ept": ept, "eot": eot,
        })
    return in_maps


def _get_compiled():
    global _COMPILED
    if _COMPILED is None:
        _COMPILED = _build_nc()
    return _COMPILED


def _unshard(results):
    out4 = np.zeros((NCH, CH, DB, 128), np.float32)
    for c in range(NCORES):
        r = np.asarray(results[c]["out_rows"]).astype(np.float32)
        r = r.reshape(NCH, 16, DB, CH)
        out4[:, :, :, 16 * c:16 * c + 16] = r.transpose(0, 3, 2, 1)
    return out4.reshape(B, T, D)


def kernel(**inputs):
    nc = _get_compiled()
    in_maps = _prep_inputs(inputs)
    res = run_bass_kernel_spmd(nc, in_maps, list(range(NCORES)))
    return _unshard(res.results)
